# revision 59
# baseline (speedup 1.0000x reference)
"""8-core Trainium2 Bass kernel for causal multi-head attention (v3, fp8).

Problem: B=4, S=2048, E=1024, H=16 heads, D=64.
  y = softmax(causal(Q K^T / sqrt(D))) V, with Q/K/V/O linear projections.

Sharding (hardcoded): hybrid batch x head split over 8 cores.
  core c -> batch b = c % 4, head-group hg = c // 4 (8 heads each).
Host sums the two partial y's per batch and adds bo.

All projections run on fp8e4 DoubleRow matmuls (0.5 cycles/row, two
contraction tiles per pass) with residual splits recovering ~bf16
accuracy where the error path matters:
  * x ships as x8 + xr (both fp8, xr = fp8 residual of x).
  * Q/K: (x8 + xr) @ Wq8 with Wq scaled x16 into fp8's sweet spot; the
    1/256 compensation folds into the exp scale.  W-side quantization
    error only perturbs softmax scores (averages out except tiny-keff
    rows) so it needs no residual chain.
  * V: x8@Wv8 + xr@Wv8 + x8@Wvr (Wv scaled x16; the x16 rides through
    PV and Wo and is divided out in the final y copy).
  * PV: V approximated as V8 + R (both fp8, same PSUM accumulation);
    probabilities are written as fp8 by ScalarE into per-pair tiles
    [128, 2, W] whose slots are q-aligned so one DoubleRow rhs AP
    covers both strips; stationary V tiles are zero-padded to 96
    columns (DoubleRow stationary free dim must be a multiple of 32)
    with a ones column for the softmax denominator.
  * Wo stays bf16 (fp8 attn would need a residual pair, costing more
    than it saves).
The causal mask is a PE matmul adding -2^30 above the diagonal into the
scores PSUM: it OPENS the diagonal block's accumulation group (zeroing
it) and the scores matmul closes it.  The V projection is woven
just-in-time into head 0's attention stream and Wo s-tiles pop during
the last head, keeping TensorE busy through the Act-bound (exp) phases.
Normalization drains PSUM with one raw copy (fast bank turnaround),
then reciprocal + DRAM-round-trip broadcast + one multiply; DVE ops
may shift partition bases, so the odd head writes attn rows 64..127
directly.  y returns bf16 (x16 hot), bo is added on the host.
"""

import functools

import ml_dtypes
import numpy as np

import concourse.bacc as bacc
import concourse.mybir as mybir
import concourse.tile as tile
from concourse.bass_utils import run_bass_kernel_spmd
from concourse.masks import make_identity, make_upper_triangular

B, S, E, H, D = 4, 2048, 1024, 16, 64
NCORES = 8
HL = H // 2  # local heads per core
CL = HL * D  # 512 local channels
P = 128
QCW = 512  # q-chunk width (one PSUM bank of fp32)
F32 = mybir.dt.float32
BF16 = mybir.dt.bfloat16
FP8 = mybir.dt.float8e4
BF = ml_dtypes.bfloat16
NP8 = ml_dtypes.float8_e4m3
EO = E // P  # 8 contraction tiles for projections
CT = CL // P  # 4 c-tiles (head pairs)
WSCALE = 16.0  # host scale on Wq/Wk/Wv (and their biases)
DR = mybir.MatmulPerfMode.DoubleRow
NEG = -float(2 ** 30)
MV = 96  # padded stationary width of [V | ones | 0...]


def build_mha_core(seq: int = S):
    assert seq % QCW == 0
    NQC = seq // QCW
    NST = seq // P
    NPP = NST // 2  # k-tile pairs
    S8 = float(D) ** -0.5 / (WSCALE * WSCALE)  # exp scale (undoes w x16 on q&k)

    nc = bacc.Bacc(None, target_bir_lowering=False)
    x8_d = nc.dram_tensor("x8", [E, seq], FP8, kind="ExternalInput")
    xr_d = nc.dram_tensor("xr", [E, seq], FP8, kind="ExternalInput")
    wq8_d = nc.dram_tensor("wq8", [E, CL], FP8, kind="ExternalInput")
    wk8_d = nc.dram_tensor("wk8", [E, CL], FP8, kind="ExternalInput")
    wv8_d = nc.dram_tensor("wv8", [E, CL], FP8, kind="ExternalInput")
    wvr_d = nc.dram_tensor("wvr", [E, CL], FP8, kind="ExternalInput")
    woT_d = nc.dram_tensor("woT", [CL, E], BF16, kind="ExternalInput")
    bq_d = nc.dram_tensor("bq", [CL], F32, kind="ExternalInput")  # x16
    bk_d = nc.dram_tensor("bk", [CL], F32, kind="ExternalInput")  # x16
    bv_d = nc.dram_tensor("bv", [CL], BF16, kind="ExternalInput")  # x16
    y_d = nc.dram_tensor("y", [seq, E], BF16, kind="ExternalOutput")  # x16
    y2_d = nc.dram_tensor("y2", [seq, E], BF16, kind="ExternalOutput")  # x16

    with tile.TileContext(nc) as tc:
        with (
            tc.tile_pool(name="singles", bufs=1) as singles,
            tc.tile_pool(name="exp_pool", bufs=9) as exp_pool,
            tc.tile_pool(name="yt_pool", bufs=7) as yt_pool,
            tc.tile_pool(name="small1", bufs=4) as small1,
            tc.tile_pool(name="dram", bufs=1, space="DRAM") as dram_pool,
            tc.tile_pool(name="psum_main", bufs=3, space="PSUM") as psum_main,
            tc.tile_pool(name="psum_acc", bufs=1, space="PSUM") as psum_acc,
        ):
            # ---------- constants ----------
            aux = singles.tile([1, P + CL], BF16)  # [ones(P) | 16*bv(CL)]
            ones_sb = aux[:, :P]
            bv_sb = aux[:, P : P + CL]
            nc.vector.memset(ones_sb, 1.0)
            nc.sync.dma_start(bv_sb, bv_d[None, :])
            # causal-mask pair: scores_psum := negI^T @ lowtri + scores
            negI_sb = singles.tile([P, P], BF16)
            make_identity(nc, negI_sb[:])
            nc.vector.tensor_scalar_mul(negI_sb[:], negI_sb[:], NEG)
            lowtri_sb = singles.tile([P, P], BF16)
            make_upper_triangular(nc, lowtri_sb[:], val=-1.0, diag=True)
            nc.vector.tensor_scalar_add(lowtri_sb[:], lowtri_sb[:], 1.0)

            negrow_sb = singles.tile([1, P], BF16)
            nc.vector.memset(negrow_sb[:], NEG)

            bqk_sb = singles.tile([P, 2, CT], F32)
            nc.sync.dma_start(bqk_sb[:, 0], bq_d[:].rearrange("(ct p) -> p ct", p=P))
            nc.sync.dma_start(bqk_sb[:, 1], bk_d[:].rearrange("(ct p) -> p ct", p=P))

            # ---------- SBUF residents (x chunked along s for fast start) ----
            wq8_sb = singles.tile([P, EO, CL], FP8)
            wk8_sb = singles.tile([P, EO, CL], FP8)
            wv8_sb = singles.tile([P, EO, CL], FP8)
            wvr_sb = singles.tile([P, EO, CL], FP8)
            x8_sb = singles.tile([P, EO, seq], FP8)
            xr_sb = singles.tile([P, EO, seq], FP8)
            wo_sb = singles.tile([P, CT, E], BF16)
            # batched DMAs (HWDGE dispatch is ~fixed cost per copy): one per
            # weight tensor, one per s-chunk for x8/xr, ordered so the QK
            # projection of pair 0 can start as early as possible
            for w_sb, w_d in ((wq8_sb, wq8_d), (wk8_sb, wk8_d)):
                nc.sync.dma_start(
                    w_sb[:], w_d[:].rearrange("(eo p) c -> p eo c", p=P)
                )
            x8_ap = x8_d[:].rearrange("(eo p) s -> p eo s", p=P)
            xr_ap = xr_d[:].rearrange("(eo p) s -> p eo s", p=P)
            for sc in range(NQC):
                nc.sync.dma_start(
                    x8_sb[:, :, sc * QCW : (sc + 1) * QCW],
                    x8_ap[:, :, sc * QCW : (sc + 1) * QCW],
                )
            for w_sb, w_d in ((wv8_sb, wv8_d), (wvr_sb, wvr_d)):
                nc.sync.dma_start(
                    w_sb[:], w_d[:].rearrange("(eo p) c -> p eo c", p=P)
                )
            for sc in range(NQC):
                nc.sync.dma_start(
                    xr_sb[:, :, sc * QCW : (sc + 1) * QCW],
                    xr_ap[:, :, sc * QCW : (sc + 1) * QCW],
                )
            nc.sync.dma_start(
                wo_sb[:], woT_d[:].rearrange("(ct p) e -> p ct e", p=P)
            )

            # per-pair Q^T/K^T tiles (bf16, x16 scale)
            qT_sb = [singles.tile([P, seq], BF16, name=f"qT{i}") for i in range(CT)]
            kT_sb = [singles.tile([P, seq], BF16, name=f"kT{i}") for i in range(CT)]
            # V (x16) as fp8 + fp8 residual, ones column at D, zero-padded
            v8_sb = singles.tile([P, NST, HL, MV], FP8)
            r_sb = singles.tile([P, NST, HL, MV], FP8)
            nc.gpsimd.memset(v8_sb[:, :, :, D:MV], 0.0)
            nc.gpsimd.memset(v8_sb[:, :, :, D : D + 1], 1.0)
            nc.gpsimd.memset(r_sb[:, :, :, D:MV], 0.0)
            attn_sb = singles.tile([P, CT, seq], BF16)
            rec_dram = dram_pool.tile([HL, seq], F32)

            # ---------- emission helpers ----------
            def emit_v_step(st):
                """V projection (x16, 3 fp8 DoubleRow chains) for one s-tile
                -> v8/r fp8 pair."""
                ps = psum_main.tile([P, 2 * QCW], F32, tag="mm", name="v_ps")
                ps = ps[:, :QCW]
                chains = (
                    (x8_sb, wv8_sb), (xr_sb, wv8_sb), (x8_sb, wvr_sb)
                )
                for ci, (xs, ws) in enumerate(chains):
                    for e in range(EO // 2):
                        nc.tensor.matmul(
                            ps[:],
                            xs[:, 2 * e : 2 * e + 2, st * P : (st + 1) * P],
                            ws[:, 2 * e : 2 * e + 2, :],
                            start=(ci == 0 and e == 0),
                            stop=False,
                            perf_mode=DR,
                        )
                nc.tensor.matmul(
                    ps[:], ones_sb[:, :P], bv_sb, start=False, stop=True
                )
                psv = ps[:].rearrange("p (h d) -> p h d", d=D)
                nc.vector.tensor_copy(v8_sb[:, st, :, 0:D], psv)
                nc.vector.tensor_sub(r_sb[:, st, :, 0:D], psv, v8_sb[:, st, :, 0:D])

            def qk_steps(pair):
                for sc in range(NQC):
                    for which, w_sb, outT in ((0, wq8_sb, qT_sb), (1, wk8_sb, kT_sb)):
                        yield which, w_sb, outT, sc

            def emit_qk_step(step, pair):
                which, w_sb, outT, sc = step
                ps = psum_main.tile([P, 2 * QCW], F32, tag="mm", name="qk_ps")
                ps = ps[:, :QCW]
                for e in range(EO // 2):
                    nc.tensor.matmul(
                        ps[:],
                        w_sb[:, 2 * e : 2 * e + 2, pair * P : (pair + 1) * P],
                        x8_sb[:, 2 * e : 2 * e + 2, sc * QCW : (sc + 1) * QCW],
                        start=(e == 0),
                        stop=(e == EO // 2 - 1),
                        perf_mode=DR,
                    )
                nc.vector.tensor_scalar_add(
                    outT[pair][:, sc * QCW : (sc + 1) * QCW],
                    ps[:],
                    bqk_sb[:, which, pair : pair + 1],
                )

            def emit_wo(st, half):
                """Half output projection (ct pair `half`) for one 128-row
                s-tile; the two halves sum on the host.  y is x16-hot and
                divided in the yt copy -- on DVE for the early half
                (mid-stream) and on Act for the late half (Act-idle tail)."""
                ps = psum_main.tile([P, 2 * QCW], F32, tag="mm", name="wo_ps")
                for ec in range(E // QCW):
                    for ci, ct in enumerate((2 * half, 2 * half + 1)):
                        nc.tensor.matmul(
                            ps[:, ec * QCW : (ec + 1) * QCW],
                            attn_sb[:, ct, st * P : (st + 1) * P],
                            wo_sb[:, ct, ec * QCW : (ec + 1) * QCW],
                            start=(ci == 0),
                            stop=(ci == 1),
                        )
                yt = yt_pool.tile([P, E], BF16, tag="yt")
                if half == 0 or st % 2 == 0:
                    nc.vector.tensor_scalar_mul(yt[:], ps[:], 1.0 / WSCALE)
                else:
                    nc.scalar.mul(yt[:], ps[:], 1.0 / WSCALE)
                yd = y_d if half == 0 else y2_d
                nc.sync.dma_start(yd[st * P : (st + 1) * P, :], yt[:])

            # ---------- attention ----------
            # pair 0's Q/K projected up front; later pairs interleave
            for step in qk_steps(0):
                emit_qk_step(step, 0)

            for pair in range(CT):
                nxt = iter(qk_steps(pair + 1)) if pair + 1 < CT else iter(())
                v_pending = list(range(NST)) if pair == 0 else []
                for hh in range(2):
                    h = 2 * pair + hh
                    hp = hh * 64
                    # po chunks live two-to-a-time in 2 PSUM banks: qc0/1
                    # accumulate inline; qc2/3 jobs are deferred and replayed
                    # from the (live) et pair tiles once the banks free up.
                    # This leaves 6 banks for a 3-deep scores rotation so
                    # V/Wo/QK filler tiles never stall the exp feed.
                    pv_total = [0] * NQC
                    for pp in range(NPP):
                        for qc in range(NQC):
                            if max(qc * QCW, 2 * pp * P) < (qc + 1) * QCW:
                                pv_total[qc] += 2
                    pv_done = [0] * NQC
                    po_tiles = {}

                    def get_po(qc):
                        if qc not in po_tiles:
                            po_tiles[qc] = psum_acc.tile(
                                [P, QCW], F32, tag=f"po{qc % 2}", name=f"po{qc}"
                            )
                        return po_tiles[qc]

                    def pv_mm(qc, out_slice, lhsT, rhs, start, perf_mode=None):
                        # start=True must zero every region on its first
                        # write: all pair-0 v8-chain matmuls open their own
                        # column range; everything else accumulates
                        pv_done[qc] += 1
                        nc.tensor.matmul(
                            out_slice,
                            lhsT,
                            rhs,
                            start=start,
                            stop=(pv_done[qc] == pv_total[qc]),
                            perf_mode=perf_mode,
                        )
                        if pv_done[qc] == pv_total[qc]:
                            _normalize_chunk(
                                nc, h, hp, pair, qc, po_tiles[qc],
                                attn_sb, rec_dram, small1,
                            )
                            if h == 3:
                                wo_early.extend(
                                    range(qc * (QCW // P), (qc + 1) * (QCW // P))
                                )
                            elif h == HL - 1:
                                wo_late.extend(
                                    range(qc * (QCW // P), (qc + 1) * (QCW // P))
                                )

                    def pv_jobs_for(ppi, qc):
                        """All PV matmuls of source pair ppi into chunk qc.
                        The kt0 leading block rides inside the DoubleRow pass:
                        et slot 1's leading 128 columns are zeroed at tile
                        allocation so slot 1 contributes nothing there."""
                        kt0 = 2 * ppi
                        tile_po = get_po(qc)
                        qs = max(qc * QCW, kt0 * P)
                        qe = (qc + 1) * QCW
                        if qs < qe:
                            j0 = qs - kt0 * P
                            w = qe - qs
                            for vt in (v8_sb, r_sb):
                                pv_mm(
                                    qc,
                                    tile_po[0:MV, qs - qc * QCW :],
                                    vt[:, kt0 : kt0 + 2, h, :],
                                    ets[ppi][:, :, j0 : j0 + w],
                                    start=(ppi == 0 and vt is v8_sb),
                                    perf_mode=DR,
                                )

                    def emit_strip(pp, sl, et2):
                        """Scores strip kt=2pp+sl -> exp(fp8) into slot sl,
                        q-aligned at j = q - 2pp*128 (slot offset sl*128)."""
                        kt = 2 * pp + sl
                        kq0 = kt * P
                        base = sl * P
                        W = seq - kq0
                        pos = 0
                        while pos < W:
                            cw = min(2 * QCW, W - pos)
                            ps = psum_main.tile(
                                [P, 2 * QCW], F32, tag="mm", name="sc_ps"
                            )
                            # independent PSUM accumulation groups per
                            # region; on the first chunk the causal-mask add
                            # OPENS the diagonal block's group (start=True
                            # zeroes it) and the scores matmul closes it --
                            # a trailing start=False matmul after an open
                            # group breaks downstream read ordering
                            if pos == 0:
                                regions = [(0, min(P, cw)), (P, QCW), (QCW, 2 * QCW)]
                            else:
                                regions = [(0, QCW), (QCW, 2 * QCW)]
                            for j0, j1 in regions:
                                jw = min(j1, cw) - j0
                                if jw <= 0:
                                    continue
                                first = pos == 0 and j0 == 0
                                if first:
                                    nc.tensor.matmul(
                                        ps[:, 0:P],
                                        negI_sb[:],
                                        lowtri_sb[:],
                                        start=True,
                                        stop=False,
                                    )
                                nc.tensor.matmul(
                                    ps[:, j0 : j0 + jw],
                                    kT_sb[pair][hp : hp + D, kq0 : kq0 + P],
                                    qT_sb[pair][
                                        hp : hp + D,
                                        kq0 + pos + j0 : kq0 + pos + j0 + jw,
                                    ],
                                    start=not first,
                                    stop=True,
                                )
                            nc.scalar.activation(
                                et2[:, sl, base + pos : base + pos + cw],
                                ps[:, :cw],
                                mybir.ActivationFunctionType.Exp,
                                scale=S8,
                            )
                            pos += cw
                        return et2

                    # software pipeline: strips(pp) issued before the
                    # inline (qc0/1) PV of pair pp-1; deferred qc2/3 jobs and
                    # fillers drain behind
                    if not (hh or pair):
                        wo_early, wo_late = [], []
                    ets = []
                    bq2 = [p for p in range(NPP)
                           if (2 * p + 1) * P < 3 * QCW or (2 * p) // (QCW // P) == 2]
                    bq3 = [p for p in range(NPP)
                           if (2 * p + 1) * P < 4 * QCW or (2 * p) // (QCW // P) == 3]
                    for pp in range(NPP + 1):
                        if pp < NPP:
                            et2 = exp_pool.tile(
                                [P, 2, seq], FP8, tag="exp", name="et2"
                            )
                            ets.append(et2)
                            if pp >= 6:
                                # narrow pair: both strips share one PSUM
                                # tile; ONE exp covers both et slots (the
                                # slot-1 leading 128 gets exp(-BIG) = 0,
                                # replacing the memset)
                                kt0 = 2 * pp
                                kq0 = kt0 * P
                                W0 = seq - kq0
                                W1 = W0 - P
                                ps = psum_main.tile(
                                    [P, 2 * QCW], F32, tag="mm", name="ms_ps"
                                )
                                # slot 0: mask opens diag, scores close; rest
                                nc.tensor.matmul(
                                    ps[:, 0:P], negI_sb[:], lowtri_sb[:],
                                    start=True, stop=False,
                                )
                                nc.tensor.matmul(
                                    ps[:, 0:P],
                                    kT_sb[pair][hp : hp + D, kq0 : kq0 + P],
                                    qT_sb[pair][hp : hp + D, kq0 : kq0 + P],
                                    start=False, stop=True,
                                )
                                if W0 > P:
                                    nc.tensor.matmul(
                                        ps[:, P:W0],
                                        kT_sb[pair][hp : hp + D, kq0 : kq0 + P],
                                        qT_sb[pair][
                                            hp : hp + D, kq0 + P : kq0 + W0
                                        ],
                                        start=True, stop=True,
                                    )
                                # slot 1 at ps[QCW:]: -BIG leading fill,
                                # then mask+diag scores, then the rest
                                nc.tensor.matmul(
                                    ps[:, QCW : QCW + P],
                                    negrow_sb[:],
                                    ones_sb[:, :P],
                                    start=True, stop=True,
                                )
                                nc.tensor.matmul(
                                    ps[:, QCW + P : QCW + 2 * P],
                                    negI_sb[:], lowtri_sb[:],
                                    start=True, stop=False,
                                )
                                nc.tensor.matmul(
                                    ps[:, QCW + P : QCW + 2 * P],
                                    kT_sb[pair][
                                        hp : hp + D, kq0 + P : kq0 + 2 * P
                                    ],
                                    qT_sb[pair][
                                        hp : hp + D, kq0 + P : kq0 + 2 * P
                                    ],
                                    start=False, stop=True,
                                )
                                if W1 > P:
                                    nc.tensor.matmul(
                                        ps[:, QCW + 2 * P : QCW + P + W1],
                                        kT_sb[pair][
                                            hp : hp + D, kq0 + P : kq0 + 2 * P
                                        ],
                                        qT_sb[pair][
                                            hp : hp + D, kq0 + 2 * P : kq0 + W0
                                        ],
                                        start=True, stop=True,
                                    )
                                nc.scalar.activation(
                                    et2[:, :, 0:W0],
                                    ps[:].rearrange("p (two w) -> p two w", two=2)[
                                        :, :, 0:W0
                                    ],
                                    mybir.ActivationFunctionType.Exp,
                                    scale=S8,
                                )
                            else:
                                nc.gpsimd.memset(et2[:, 1, 0:P], 0.0)
                                emit_strip(pp, 0, et2)
                                emit_strip(pp, 1, et2)
                            # weave V projection (pair 0 only): PV(pp) needs
                            # v8/r k-tiles 2pp..2pp+1
                            while v_pending and v_pending[0] <= 2 * pp + 1:
                                emit_v_step(v_pending.pop(0))
                        if pp >= 1:
                            ppi = pp - 1
                            for qc in (0, 1):
                                if ppi <= 2 * qc + 1:
                                    pv_jobs_for(ppi, qc)
                            for _ in range(2):
                                step = next(nxt, None)
                                if step is None:
                                    break
                                emit_qk_step(step, pair + 1)
                            for qc, queue, t0 in ((2, bq2, 3), (3, bq3, 5)):
                                n = 0
                                while (
                                    pp >= t0 and queue and queue[0] <= pp - 1
                                    and n < 2
                                ):
                                    pv_jobs_for(queue.pop(0), qc)
                                    n += 1
                            pops = 0
                            if pair >= 2 and wo_early:
                                # early half (cts 0-1), ready since pair 1;
                                # 1/iter spreads it across pairs 2-3
                                emit_wo(wo_early.pop(0), 0)
                                pops += 1
                            while wo_late and pops < 2 and pp >= 3:
                                emit_wo(wo_late.pop(0), 1)
                                pops += 1
                    for qc, queue in ((2, bq2), (3, bq3)):
                        while queue:
                            pv_jobs_for(queue.pop(0), qc)
                for step in nxt:  # leftovers
                    emit_qk_step(step, pair + 1)
            for st in wo_early:
                emit_wo(st, 0)
            for st in wo_late:
                emit_wo(st, 1)

    nc.compile()
    return nc


def _normalize_chunk(nc, h, hp, pair, qc, po, attn_sb, rec_dram, small1):
    """attn[c, q] = po[d, q] * (1 / sums[q]); sums live in po row D.

    One raw DVE copy drains the PSUM bank immediately (fast turnaround
    for the next head), then reciprocal (partition-base shift 64->0),
    DRAM round-trip partition broadcast, and one multiply whose output
    partition base may differ from its inputs' (odd head -> rows 64+).
    """
    q0 = qc * QCW
    pot = small1.tile([D + 1, QCW], F32, tag="pot")
    nc.vector.tensor_copy(pot[:], po[0 : D + 1, :])
    srow = small1.tile([1, QCW], F32, tag="srow")
    nc.vector.reciprocal(srow[0:1, :], pot[D : D + 1, :])
    nc.sync.dma_start(rec_dram[h, q0 : q0 + QCW], srow[0:1, :])
    rb = small1.tile([D, QCW], F32, tag="rb")
    nc.sync.dma_start(
        rb[:],
        rec_dram[h, q0 : q0 + QCW][None, :].to_broadcast((D, QCW)),
    )
    nc.vector.tensor_mul(
        attn_sb[hp : hp + D, pair, q0 : q0 + QCW], pot[0:D, :], rb[:]
    )


@functools.lru_cache(maxsize=2)
def _get_nc(seq: int):
    return build_mha_core(seq)


def make_in_maps(x, Wq, bq, Wk, bk, Wv, bv, Wo, bo, seq: int = S):
    """Shard + pre-layout the full inputs for the 8 cores."""

    def bf(a):
        return np.ascontiguousarray(a.astype(BF))

    def f8(a):
        return np.ascontiguousarray(a.astype(NP8))

    in_maps = []
    for c in range(NCORES):
        b, hg = c % 4, c // 4
        cs = slice(hg * CL, (hg + 1) * CL)
        xT = x[b][:seq].T
        x8 = xT.astype(NP8)
        xr = (xT - x8.astype(np.float32)).astype(NP8)
        wv16 = WSCALE * Wv[cs, :].T
        wv8 = wv16.astype(NP8)
        wvr = (wv16 - wv8.astype(np.float32)).astype(NP8)
        in_maps.append(
            {
                "x8": x8,
                "xr": xr,
                "wq8": f8(WSCALE * Wq[cs, :].T),
                "wk8": f8(WSCALE * Wk[cs, :].T),
                "wv8": wv8,
                "wvr": wvr,
                "woT": bf(Wo[:, cs].T),
                "bq": np.ascontiguousarray(WSCALE * bq[cs], dtype=np.float32),
                "bk": np.ascontiguousarray(WSCALE * bk[cs], dtype=np.float32),
                "bv": bf(WSCALE * bv[cs]),
            }
        )
    return in_maps


def kernel(x, Wq, bq, Wk, bk, Wv, bv, Wo, bo, _trace: bool = False):
    x = np.asarray(x, np.float32)
    args = [np.asarray(a, np.float32) for a in (Wq, bq, Wk, bk, Wv, bv, Wo, bo)]
    nc = _get_nc(S)
    in_maps = make_in_maps(x, *args)
    try:
        res = run_bass_kernel_spmd(
            nc, in_maps, core_ids=list(range(NCORES)), trace=_trace
        )
    except ModuleNotFoundError:
        res = run_bass_kernel_spmd(nc, in_maps, core_ids=list(range(NCORES)))
    outs = res.results
    bo32 = np.asarray(bo, np.float32)
    y = np.empty((B, S, E), np.float32)
    for b in range(B):
        y[b] = (
            outs[b]["y"].astype(np.float32)
            + outs[b]["y2"].astype(np.float32)
            + outs[b + 4]["y"].astype(np.float32)
            + outs[b + 4]["y2"].astype(np.float32)
            + bo32
        )
    kernel.last_exec_time_ns = res.exec_time_ns
    kernel.last_results = res
    return y


# revision 60
# speedup vs baseline: 1.0004x; 1.0004x over previous
"""8-core Trainium2 Bass kernel for causal multi-head attention (v3, fp8).

Problem: B=4, S=2048, E=1024, H=16 heads, D=64.
  y = softmax(causal(Q K^T / sqrt(D))) V, with Q/K/V/O linear projections.

Sharding (hardcoded): hybrid batch x head split over 8 cores.
  core c -> batch b = c % 4, head-group hg = c // 4 (8 heads each).
Host sums the two partial y's per batch and adds bo.

All projections run on fp8e4 DoubleRow matmuls (0.5 cycles/row, two
contraction tiles per pass) with residual splits recovering ~bf16
accuracy where the error path matters:
  * x ships as x8 + xr (both fp8, xr = fp8 residual of x).
  * Q/K: (x8 + xr) @ Wq8 with Wq scaled x16 into fp8's sweet spot; the
    1/256 compensation folds into the exp scale.  W-side quantization
    error only perturbs softmax scores (averages out except tiny-keff
    rows) so it needs no residual chain.
  * V: x8@Wv8 + xr@Wv8 + x8@Wvr (Wv scaled x16; the x16 rides through
    PV and Wo and is divided out in the final y copy).
  * PV: V approximated as V8 + R (both fp8, same PSUM accumulation);
    probabilities are written as fp8 by ScalarE into per-pair tiles
    [128, 2, W] whose slots are q-aligned so one DoubleRow rhs AP
    covers both strips; stationary V tiles are zero-padded to 96
    columns (DoubleRow stationary free dim must be a multiple of 32)
    with a ones column for the softmax denominator.
  * Wo stays bf16 (fp8 attn would need a residual pair, costing more
    than it saves).
The causal mask is a PE matmul adding -2^30 above the diagonal into the
scores PSUM: it OPENS the diagonal block's accumulation group (zeroing
it) and the scores matmul closes it.  The V projection is woven
just-in-time into head 0's attention stream and Wo s-tiles pop during
the last head, keeping TensorE busy through the Act-bound (exp) phases.
Normalization drains PSUM with one raw copy (fast bank turnaround),
then reciprocal + DRAM-round-trip broadcast + one multiply; DVE ops
may shift partition bases, so the odd head writes attn rows 64..127
directly.  y returns bf16 (x16 hot), bo is added on the host.
"""

import functools

import ml_dtypes
import numpy as np

import concourse.bacc as bacc
import concourse.mybir as mybir
import concourse.tile as tile
from concourse.bass_utils import run_bass_kernel_spmd
from concourse.masks import make_identity, make_upper_triangular

B, S, E, H, D = 4, 2048, 1024, 16, 64
NCORES = 8
HL = H // 2  # local heads per core
CL = HL * D  # 512 local channels
P = 128
QCW = 512  # q-chunk width (one PSUM bank of fp32)
F32 = mybir.dt.float32
BF16 = mybir.dt.bfloat16
FP8 = mybir.dt.float8e4
BF = ml_dtypes.bfloat16
NP8 = ml_dtypes.float8_e4m3
EO = E // P  # 8 contraction tiles for projections
CT = CL // P  # 4 c-tiles (head pairs)
WSCALE = 16.0  # host scale on Wq/Wk/Wv (and their biases)
DR = mybir.MatmulPerfMode.DoubleRow
NEG = -float(2 ** 30)
MV = 96  # padded stationary width of [V | ones | 0...]


def build_mha_core(seq: int = S):
    assert seq % QCW == 0
    NQC = seq // QCW
    NST = seq // P
    NPP = NST // 2  # k-tile pairs
    S8 = float(D) ** -0.5 / (WSCALE * WSCALE)  # exp scale (undoes w x16 on q&k)

    nc = bacc.Bacc(None, target_bir_lowering=False)
    x8_d = nc.dram_tensor("x8", [E, seq], FP8, kind="ExternalInput")
    xr_d = nc.dram_tensor("xr", [E, seq], FP8, kind="ExternalInput")
    wq8_d = nc.dram_tensor("wq8", [E, CL], FP8, kind="ExternalInput")
    wk8_d = nc.dram_tensor("wk8", [E, CL], FP8, kind="ExternalInput")
    wv8_d = nc.dram_tensor("wv8", [E, CL], FP8, kind="ExternalInput")
    wvr_d = nc.dram_tensor("wvr", [E, CL], FP8, kind="ExternalInput")
    woT_d = nc.dram_tensor("woT", [CL, E], BF16, kind="ExternalInput")
    bq_d = nc.dram_tensor("bq", [CL], F32, kind="ExternalInput")  # x16
    bk_d = nc.dram_tensor("bk", [CL], F32, kind="ExternalInput")  # x16
    bv_d = nc.dram_tensor("bv", [CL], BF16, kind="ExternalInput")  # x16
    y_d = nc.dram_tensor("y", [seq, E], BF16, kind="ExternalOutput")  # x16
    y2_d = nc.dram_tensor("y2", [seq, E], BF16, kind="ExternalOutput")  # x16

    with tile.TileContext(nc) as tc:
        with (
            tc.tile_pool(name="singles", bufs=1) as singles,
            tc.tile_pool(name="exp_pool", bufs=10) as exp_pool,
            tc.tile_pool(name="yt_pool", bufs=6) as yt_pool,
            tc.tile_pool(name="small1", bufs=4) as small1,
            tc.tile_pool(name="dram", bufs=1, space="DRAM") as dram_pool,
            tc.tile_pool(name="psum_main", bufs=3, space="PSUM") as psum_main,
            tc.tile_pool(name="psum_acc", bufs=1, space="PSUM") as psum_acc,
        ):
            # ---------- constants ----------
            aux = singles.tile([1, P + CL], BF16)  # [ones(P) | 16*bv(CL)]
            ones_sb = aux[:, :P]
            bv_sb = aux[:, P : P + CL]
            nc.vector.memset(ones_sb, 1.0)
            nc.sync.dma_start(bv_sb, bv_d[None, :])
            # causal-mask pair: scores_psum := negI^T @ lowtri + scores
            negI_sb = singles.tile([P, P], BF16)
            make_identity(nc, negI_sb[:])
            nc.vector.tensor_scalar_mul(negI_sb[:], negI_sb[:], NEG)
            lowtri_sb = singles.tile([P, P], BF16)
            make_upper_triangular(nc, lowtri_sb[:], val=-1.0, diag=True)
            nc.vector.tensor_scalar_add(lowtri_sb[:], lowtri_sb[:], 1.0)

            negrow_sb = singles.tile([1, P], BF16)
            nc.vector.memset(negrow_sb[:], NEG)

            bqk_sb = singles.tile([P, 2, CT], F32)
            nc.sync.dma_start(bqk_sb[:, 0], bq_d[:].rearrange("(ct p) -> p ct", p=P))
            nc.sync.dma_start(bqk_sb[:, 1], bk_d[:].rearrange("(ct p) -> p ct", p=P))

            # ---------- SBUF residents (x chunked along s for fast start) ----
            wq8_sb = singles.tile([P, EO, CL], FP8)
            wk8_sb = singles.tile([P, EO, CL], FP8)
            wv8_sb = singles.tile([P, EO, CL], FP8)
            wvr_sb = singles.tile([P, EO, CL], FP8)
            x8_sb = singles.tile([P, EO, seq], FP8)
            xr_sb = singles.tile([P, EO, seq], FP8)
            wo_sb = singles.tile([P, CT, E], BF16)
            # batched DMAs (HWDGE dispatch is ~fixed cost per copy): one per
            # weight tensor, one per s-chunk for x8/xr, ordered so the QK
            # projection of pair 0 can start as early as possible
            for w_sb, w_d in ((wq8_sb, wq8_d), (wk8_sb, wk8_d)):
                nc.sync.dma_start(
                    w_sb[:], w_d[:].rearrange("(eo p) c -> p eo c", p=P)
                )
            x8_ap = x8_d[:].rearrange("(eo p) s -> p eo s", p=P)
            xr_ap = xr_d[:].rearrange("(eo p) s -> p eo s", p=P)
            for sc in range(NQC):
                nc.sync.dma_start(
                    x8_sb[:, :, sc * QCW : (sc + 1) * QCW],
                    x8_ap[:, :, sc * QCW : (sc + 1) * QCW],
                )
            for w_sb, w_d in ((wv8_sb, wv8_d), (wvr_sb, wvr_d)):
                nc.sync.dma_start(
                    w_sb[:], w_d[:].rearrange("(eo p) c -> p eo c", p=P)
                )
            for sc in range(NQC):
                nc.sync.dma_start(
                    xr_sb[:, :, sc * QCW : (sc + 1) * QCW],
                    xr_ap[:, :, sc * QCW : (sc + 1) * QCW],
                )
            nc.sync.dma_start(
                wo_sb[:], woT_d[:].rearrange("(ct p) e -> p ct e", p=P)
            )

            # per-pair Q^T/K^T tiles (bf16, x16 scale)
            qT_sb = [singles.tile([P, seq], BF16, name=f"qT{i}") for i in range(CT)]
            kT_sb = [singles.tile([P, seq], BF16, name=f"kT{i}") for i in range(CT)]
            # V (x16) as fp8 + fp8 residual, ones column at D, zero-padded
            v8_sb = singles.tile([P, NST, HL, MV], FP8)
            r_sb = singles.tile([P, NST, HL, MV], FP8)
            nc.gpsimd.memset(v8_sb[:, :, :, D:MV], 0.0)
            nc.gpsimd.memset(v8_sb[:, :, :, D : D + 1], 1.0)
            nc.gpsimd.memset(r_sb[:, :, :, D:MV], 0.0)
            attn_sb = singles.tile([P, CT, seq], BF16)
            rec_dram = dram_pool.tile([HL, seq], F32)

            # ---------- emission helpers ----------
            def emit_v_step(st):
                """V projection (x16, 3 fp8 DoubleRow chains) for one s-tile
                -> v8/r fp8 pair."""
                ps = psum_main.tile([P, 2 * QCW], F32, tag="mm", name="v_ps")
                ps = ps[:, :QCW]
                chains = (
                    (x8_sb, wv8_sb), (xr_sb, wv8_sb), (x8_sb, wvr_sb)
                )
                for ci, (xs, ws) in enumerate(chains):
                    for e in range(EO // 2):
                        nc.tensor.matmul(
                            ps[:],
                            xs[:, 2 * e : 2 * e + 2, st * P : (st + 1) * P],
                            ws[:, 2 * e : 2 * e + 2, :],
                            start=(ci == 0 and e == 0),
                            stop=False,
                            perf_mode=DR,
                        )
                nc.tensor.matmul(
                    ps[:], ones_sb[:, :P], bv_sb, start=False, stop=True
                )
                psv = ps[:].rearrange("p (h d) -> p h d", d=D)
                nc.vector.tensor_copy(v8_sb[:, st, :, 0:D], psv)
                nc.vector.tensor_sub(r_sb[:, st, :, 0:D], psv, v8_sb[:, st, :, 0:D])

            def qk_steps(pair):
                for sc in range(NQC):
                    for which, w_sb, outT in ((0, wq8_sb, qT_sb), (1, wk8_sb, kT_sb)):
                        yield which, w_sb, outT, sc

            def emit_qk_step(step, pair):
                which, w_sb, outT, sc = step
                ps = psum_main.tile([P, 2 * QCW], F32, tag="mm", name="qk_ps")
                ps = ps[:, :QCW]
                for e in range(EO // 2):
                    nc.tensor.matmul(
                        ps[:],
                        w_sb[:, 2 * e : 2 * e + 2, pair * P : (pair + 1) * P],
                        x8_sb[:, 2 * e : 2 * e + 2, sc * QCW : (sc + 1) * QCW],
                        start=(e == 0),
                        stop=(e == EO // 2 - 1),
                        perf_mode=DR,
                    )
                nc.vector.tensor_scalar_add(
                    outT[pair][:, sc * QCW : (sc + 1) * QCW],
                    ps[:],
                    bqk_sb[:, which, pair : pair + 1],
                )

            def emit_wo(st, half):
                """Half output projection (ct pair `half`) for one 128-row
                s-tile; the two halves sum on the host.  y is x16-hot and
                divided in the yt copy -- on DVE for the early half
                (mid-stream) and on Act for the late half (Act-idle tail)."""
                ps = psum_main.tile([P, 2 * QCW], F32, tag="mm", name="wo_ps")
                for ec in range(E // QCW):
                    for ci, ct in enumerate((2 * half, 2 * half + 1)):
                        nc.tensor.matmul(
                            ps[:, ec * QCW : (ec + 1) * QCW],
                            attn_sb[:, ct, st * P : (st + 1) * P],
                            wo_sb[:, ct, ec * QCW : (ec + 1) * QCW],
                            start=(ci == 0),
                            stop=(ci == 1),
                        )
                yt = yt_pool.tile([P, E], BF16, tag="yt")
                if half == 0 or st % 2 == 0:
                    nc.vector.tensor_scalar_mul(yt[:], ps[:], 1.0 / WSCALE)
                else:
                    nc.scalar.mul(yt[:], ps[:], 1.0 / WSCALE)
                yd = y_d if half == 0 else y2_d
                nc.sync.dma_start(yd[st * P : (st + 1) * P, :], yt[:])

            # ---------- attention ----------
            # pair 0's Q/K projected up front; later pairs interleave
            for step in qk_steps(0):
                emit_qk_step(step, 0)

            for pair in range(CT):
                nxt = iter(qk_steps(pair + 1)) if pair + 1 < CT else iter(())
                v_pending = list(range(NST)) if pair == 0 else []
                for hh in range(2):
                    h = 2 * pair + hh
                    hp = hh * 64
                    # po chunks live two-to-a-time in 2 PSUM banks: qc0/1
                    # accumulate inline; qc2/3 jobs are deferred and replayed
                    # from the (live) et pair tiles once the banks free up.
                    # This leaves 6 banks for a 3-deep scores rotation so
                    # V/Wo/QK filler tiles never stall the exp feed.
                    pv_total = [0] * NQC
                    for pp in range(NPP):
                        for qc in range(NQC):
                            if max(qc * QCW, 2 * pp * P) < (qc + 1) * QCW:
                                pv_total[qc] += 2
                    pv_done = [0] * NQC
                    po_tiles = {}

                    def get_po(qc):
                        if qc not in po_tiles:
                            po_tiles[qc] = psum_acc.tile(
                                [P, QCW], F32, tag=f"po{qc % 2}", name=f"po{qc}"
                            )
                        return po_tiles[qc]

                    def pv_mm(qc, out_slice, lhsT, rhs, start, perf_mode=None):
                        # start=True must zero every region on its first
                        # write: all pair-0 v8-chain matmuls open their own
                        # column range; everything else accumulates
                        pv_done[qc] += 1
                        nc.tensor.matmul(
                            out_slice,
                            lhsT,
                            rhs,
                            start=start,
                            stop=(pv_done[qc] == pv_total[qc]),
                            perf_mode=perf_mode,
                        )
                        if pv_done[qc] == pv_total[qc]:
                            _normalize_chunk(
                                nc, h, hp, pair, qc, po_tiles[qc],
                                attn_sb, rec_dram, small1,
                            )
                            if h == 3:
                                wo_early.extend(
                                    range(qc * (QCW // P), (qc + 1) * (QCW // P))
                                )
                            elif h == HL - 1:
                                wo_late.extend(
                                    range(qc * (QCW // P), (qc + 1) * (QCW // P))
                                )

                    def pv_jobs_for(ppi, qc):
                        """All PV matmuls of source pair ppi into chunk qc.
                        The kt0 leading block rides inside the DoubleRow pass:
                        et slot 1's leading 128 columns are zeroed at tile
                        allocation so slot 1 contributes nothing there."""
                        kt0 = 2 * ppi
                        tile_po = get_po(qc)
                        qs = max(qc * QCW, kt0 * P)
                        qe = (qc + 1) * QCW
                        if qs < qe:
                            j0 = qs - kt0 * P
                            w = qe - qs
                            for vt in (v8_sb, r_sb):
                                pv_mm(
                                    qc,
                                    tile_po[0:MV, qs - qc * QCW :],
                                    vt[:, kt0 : kt0 + 2, h, :],
                                    ets[ppi][:, :, j0 : j0 + w],
                                    start=(ppi == 0 and vt is v8_sb),
                                    perf_mode=DR,
                                )

                    def emit_strip(pp, sl, et2):
                        """Scores strip kt=2pp+sl -> exp(fp8) into slot sl,
                        q-aligned at j = q - 2pp*128 (slot offset sl*128)."""
                        kt = 2 * pp + sl
                        kq0 = kt * P
                        base = sl * P
                        W = seq - kq0
                        pos = 0
                        while pos < W:
                            cw = min(2 * QCW, W - pos)
                            ps = psum_main.tile(
                                [P, 2 * QCW], F32, tag="mm", name="sc_ps"
                            )
                            # independent PSUM accumulation groups per
                            # region; on the first chunk the causal-mask add
                            # OPENS the diagonal block's group (start=True
                            # zeroes it) and the scores matmul closes it --
                            # a trailing start=False matmul after an open
                            # group breaks downstream read ordering
                            if pos == 0:
                                regions = [(0, min(P, cw)), (P, QCW), (QCW, 2 * QCW)]
                            else:
                                regions = [(0, QCW), (QCW, 2 * QCW)]
                            for j0, j1 in regions:
                                jw = min(j1, cw) - j0
                                if jw <= 0:
                                    continue
                                first = pos == 0 and j0 == 0
                                if first:
                                    nc.tensor.matmul(
                                        ps[:, 0:P],
                                        negI_sb[:],
                                        lowtri_sb[:],
                                        start=True,
                                        stop=False,
                                    )
                                nc.tensor.matmul(
                                    ps[:, j0 : j0 + jw],
                                    kT_sb[pair][hp : hp + D, kq0 : kq0 + P],
                                    qT_sb[pair][
                                        hp : hp + D,
                                        kq0 + pos + j0 : kq0 + pos + j0 + jw,
                                    ],
                                    start=not first,
                                    stop=True,
                                )
                            nc.scalar.activation(
                                et2[:, sl, base + pos : base + pos + cw],
                                ps[:, :cw],
                                mybir.ActivationFunctionType.Exp,
                                scale=S8,
                            )
                            pos += cw
                        return et2

                    # software pipeline: strips(pp) issued before the
                    # inline (qc0/1) PV of pair pp-1; deferred qc2/3 jobs and
                    # fillers drain behind
                    if not (hh or pair):
                        wo_early, wo_late = [], []
                    ets = []
                    bq2 = [p for p in range(NPP)
                           if (2 * p + 1) * P < 3 * QCW or (2 * p) // (QCW // P) == 2]
                    bq3 = [p for p in range(NPP)
                           if (2 * p + 1) * P < 4 * QCW or (2 * p) // (QCW // P) == 3]
                    for pp in range(NPP + 1):
                        if pp < NPP:
                            et2 = exp_pool.tile(
                                [P, 2, seq], FP8, tag="exp", name="et2"
                            )
                            ets.append(et2)
                            if pp >= 6:
                                # narrow pair: both strips share one PSUM
                                # tile; ONE exp covers both et slots (the
                                # slot-1 leading 128 gets exp(-BIG) = 0,
                                # replacing the memset)
                                kt0 = 2 * pp
                                kq0 = kt0 * P
                                W0 = seq - kq0
                                W1 = W0 - P
                                ps = psum_main.tile(
                                    [P, 2 * QCW], F32, tag="mm", name="ms_ps"
                                )
                                # slot 0: mask opens diag, scores close; rest
                                nc.tensor.matmul(
                                    ps[:, 0:P], negI_sb[:], lowtri_sb[:],
                                    start=True, stop=False,
                                )
                                nc.tensor.matmul(
                                    ps[:, 0:P],
                                    kT_sb[pair][hp : hp + D, kq0 : kq0 + P],
                                    qT_sb[pair][hp : hp + D, kq0 : kq0 + P],
                                    start=False, stop=True,
                                )
                                if W0 > P:
                                    nc.tensor.matmul(
                                        ps[:, P:W0],
                                        kT_sb[pair][hp : hp + D, kq0 : kq0 + P],
                                        qT_sb[pair][
                                            hp : hp + D, kq0 + P : kq0 + W0
                                        ],
                                        start=True, stop=True,
                                    )
                                # slot 1 at ps[QCW:]: -BIG leading fill,
                                # then mask+diag scores, then the rest
                                nc.tensor.matmul(
                                    ps[:, QCW : QCW + P],
                                    negrow_sb[:],
                                    ones_sb[:, :P],
                                    start=True, stop=True,
                                )
                                nc.tensor.matmul(
                                    ps[:, QCW + P : QCW + 2 * P],
                                    negI_sb[:], lowtri_sb[:],
                                    start=True, stop=False,
                                )
                                nc.tensor.matmul(
                                    ps[:, QCW + P : QCW + 2 * P],
                                    kT_sb[pair][
                                        hp : hp + D, kq0 + P : kq0 + 2 * P
                                    ],
                                    qT_sb[pair][
                                        hp : hp + D, kq0 + P : kq0 + 2 * P
                                    ],
                                    start=False, stop=True,
                                )
                                if W1 > P:
                                    nc.tensor.matmul(
                                        ps[:, QCW + 2 * P : QCW + P + W1],
                                        kT_sb[pair][
                                            hp : hp + D, kq0 + P : kq0 + 2 * P
                                        ],
                                        qT_sb[pair][
                                            hp : hp + D, kq0 + 2 * P : kq0 + W0
                                        ],
                                        start=True, stop=True,
                                    )
                                nc.scalar.activation(
                                    et2[:, :, 0:W0],
                                    ps[:].rearrange("p (two w) -> p two w", two=2)[
                                        :, :, 0:W0
                                    ],
                                    mybir.ActivationFunctionType.Exp,
                                    scale=S8,
                                )
                            else:
                                nc.gpsimd.memset(et2[:, 1, 0:P], 0.0)
                                emit_strip(pp, 0, et2)
                                emit_strip(pp, 1, et2)
                            # weave V projection (pair 0 only): PV(pp) needs
                            # v8/r k-tiles 2pp..2pp+1
                            while v_pending and v_pending[0] <= 2 * pp + 1:
                                emit_v_step(v_pending.pop(0))
                        if pp >= 1:
                            ppi = pp - 1
                            for qc in (0, 1):
                                if ppi <= 2 * qc + 1:
                                    pv_jobs_for(ppi, qc)
                            for _ in range(2):
                                step = next(nxt, None)
                                if step is None:
                                    break
                                emit_qk_step(step, pair + 1)
                            for qc, queue, t0 in ((2, bq2, 3), (3, bq3, 5)):
                                n = 0
                                while (
                                    pp >= t0 and queue and queue[0] <= pp - 1
                                    and n < 2
                                ):
                                    pv_jobs_for(queue.pop(0), qc)
                                    n += 1
                            pops = 0
                            if pair >= 2 and wo_early:
                                # early half (cts 0-1), ready since pair 1;
                                # 1/iter spreads it across pairs 2-3
                                emit_wo(wo_early.pop(0), 0)
                                pops += 1
                            while wo_late and pops < 2 and pp >= 3:
                                emit_wo(wo_late.pop(0), 1)
                                pops += 1
                    for qc, queue in ((2, bq2), (3, bq3)):
                        while queue:
                            pv_jobs_for(queue.pop(0), qc)
                for step in nxt:  # leftovers
                    emit_qk_step(step, pair + 1)
            for st in wo_early:
                emit_wo(st, 0)
            for st in wo_late:
                emit_wo(st, 1)

    nc.compile()
    return nc


def _normalize_chunk(nc, h, hp, pair, qc, po, attn_sb, rec_dram, small1):
    """attn[c, q] = po[d, q] * (1 / sums[q]); sums live in po row D.

    One raw DVE copy drains the PSUM bank immediately (fast turnaround
    for the next head), then reciprocal (partition-base shift 64->0),
    DRAM round-trip partition broadcast, and one multiply whose output
    partition base may differ from its inputs' (odd head -> rows 64+).
    """
    q0 = qc * QCW
    pot = small1.tile([D + 1, QCW], F32, tag="pot")
    nc.vector.tensor_copy(pot[:], po[0 : D + 1, :])
    srow = small1.tile([1, QCW], F32, tag="srow")
    nc.vector.reciprocal(srow[0:1, :], pot[D : D + 1, :])
    nc.sync.dma_start(rec_dram[h, q0 : q0 + QCW], srow[0:1, :])
    rb = small1.tile([D, QCW], F32, tag="rb")
    nc.sync.dma_start(
        rb[:],
        rec_dram[h, q0 : q0 + QCW][None, :].to_broadcast((D, QCW)),
    )
    nc.vector.tensor_mul(
        attn_sb[hp : hp + D, pair, q0 : q0 + QCW], pot[0:D, :], rb[:]
    )


@functools.lru_cache(maxsize=2)
def _get_nc(seq: int):
    return build_mha_core(seq)


def make_in_maps(x, Wq, bq, Wk, bk, Wv, bv, Wo, bo, seq: int = S):
    """Shard + pre-layout the full inputs for the 8 cores."""

    def bf(a):
        return np.ascontiguousarray(a.astype(BF))

    def f8(a):
        return np.ascontiguousarray(a.astype(NP8))

    in_maps = []
    for c in range(NCORES):
        b, hg = c % 4, c // 4
        cs = slice(hg * CL, (hg + 1) * CL)
        xT = x[b][:seq].T
        x8 = xT.astype(NP8)
        xr = (xT - x8.astype(np.float32)).astype(NP8)
        wv16 = WSCALE * Wv[cs, :].T
        wv8 = wv16.astype(NP8)
        wvr = (wv16 - wv8.astype(np.float32)).astype(NP8)
        in_maps.append(
            {
                "x8": x8,
                "xr": xr,
                "wq8": f8(WSCALE * Wq[cs, :].T),
                "wk8": f8(WSCALE * Wk[cs, :].T),
                "wv8": wv8,
                "wvr": wvr,
                "woT": bf(Wo[:, cs].T),
                "bq": np.ascontiguousarray(WSCALE * bq[cs], dtype=np.float32),
                "bk": np.ascontiguousarray(WSCALE * bk[cs], dtype=np.float32),
                "bv": bf(WSCALE * bv[cs]),
            }
        )
    return in_maps


def kernel(x, Wq, bq, Wk, bk, Wv, bv, Wo, bo, _trace: bool = False):
    x = np.asarray(x, np.float32)
    args = [np.asarray(a, np.float32) for a in (Wq, bq, Wk, bk, Wv, bv, Wo, bo)]
    nc = _get_nc(S)
    in_maps = make_in_maps(x, *args)
    try:
        res = run_bass_kernel_spmd(
            nc, in_maps, core_ids=list(range(NCORES)), trace=_trace
        )
    except ModuleNotFoundError:
        res = run_bass_kernel_spmd(nc, in_maps, core_ids=list(range(NCORES)))
    outs = res.results
    bo32 = np.asarray(bo, np.float32)
    y = np.empty((B, S, E), np.float32)
    for b in range(B):
        y[b] = (
            outs[b]["y"].astype(np.float32)
            + outs[b]["y2"].astype(np.float32)
            + outs[b + 4]["y"].astype(np.float32)
            + outs[b + 4]["y2"].astype(np.float32)
            + bo32
        )
    kernel.last_exec_time_ns = res.exec_time_ns
    kernel.last_results = res
    return y
